# revision 50
# baseline (speedup 1.0000x reference)
"""Self-attention (sigmoid attention) Bass kernel for Trainium2, SPMD on 8 cores.

Problem: B=4, S=1024, F=256, H=8
  q = (X @ Wq).reshape(b,s,f,h); k,v likewise (self-attention)
  attn = sigmoid(sqrt(F) * q.kT) per (b,h);  wv = attn @ v
  out = relu(wv_flat @ Wo)

Sharding: data-parallel over (batch, seq-half): core c handles batch c//2,
query rows [half*512, half*512+512). K/V are computed per-core for the full
batch sequence (duplicated across the 2 cores sharing a batch) — no
collectives needed.

Precision plan (validated vs fp32 reference, rel-max tolerance 2e-2):
  - all activations/weights bf16 (rel err 6e-3); same PE rate as fp32r but
    half the DMA/DVE/SBUF traffic and Fast Weight Load on LDWEIGHTS.
  - the attn@V reduction runs in fp8-e4m3 with MatmulPerfMode.DoubleRow
    (two j-tiles packed per matmul -> 2x ALU throughput); adds ~1e-2 rel
    err, total ~1.6e-2. PSUM accumulation is fp32 throughout.

Per-core layout (head-contiguous permuted weights, prepared on host):
  xqT  [256, 512]  X[b]^T query-half columns (features on partitions)
  xoT  [256, 512]  other-half columns
  Wq/Wk/Wv [8,256,256] (h, f_in, f_head):  W[:, f*H+h] -> [h, :, f]
  Wo   [8,256,256] (h, f_head, n):         Wo[f*H+h, :] -> [h, f, :]
Pipeline per head h: QT=[wq^T x_q], KT, V via projection matmuls;
ST_j = KT_j^T-block @ QT (psum) -> sigmoid*16 -> fp8 AT pair tiles;
OT = sum_jp Vpair_jp^T @ ATpair_jp (DoubleRow); out += OT^T @ Wo_h
accumulated in persistent PSUM banks across heads; relu at the end.
"""

import numpy as np
import ml_dtypes

B, S, F, H = 4, 1024, 256, 8
N_CORES = 8
SCALE = 16.0  # sqrt(F)
SQ = S // 2  # query rows per core

BF16 = ml_dtypes.bfloat16

N_WARM = 10  # PE clock-warmup matmuls issued during the initial DMA wait

_CACHE = {}


def _build_nc():
    import concourse.mybir as mybir
    import concourse.tile as tile
    from concourse import bacc
    from concourse.tile_rust import add_dep_helper

    f32 = mybir.dt.float32
    bdt = mybir.dt.bfloat16
    f8 = mybir.dt.float8e4
    DoubleRow = mybir.MatmulPerfMode.DoubleRow

    Sigmoid = mybir.ActivationFunctionType.Sigmoid
    Relu = mybir.ActivationFunctionType.Relu

    nc = bacc.Bacc()
    xqT = nc.declare_dram_parameter("xqT", [F, SQ], bdt, isOutput=False)
    xoT = nc.declare_dram_parameter("xoT", [F, SQ], bdt, isOutput=False)
    # weights packed per head as [128, 4*F] = [w?0 | w?1 | w?0 | w?1]
    # (kk chunks side by side) so each head needs one DMA per queue
    Wqk = nc.declare_dram_parameter("Wqk", [H, 128, 4 * F], bdt, isOutput=False)
    Wvo = nc.declare_dram_parameter("Wvo", [H, 128, 4 * F], bdt, isOutput=False)
    out_d = nc.declare_dram_parameter("out", [SQ, F], f32, isOutput=True)

    NJ = S // 128  # 8 key-row tiles per head
    NP = NJ // 2  # 4 key-row tile PAIRS (fp8 DoubleRow granularity)
    NM = SQ // 128  # 4 query-row tiles

    with tile.TileContext(nc) as tc:
        with (
            tc.tile_pool(name="const", bufs=1) as const,
            tc.tile_pool(name="sb", bufs=2) as sb,
            tc.tile_pool(name="osb", bufs=1) as osb,
            tc.tile_pool(name="psA", bufs=4, space="PSUM") as psA,
            tc.tile_pool(name="psB", bufs=2, space="PSUM") as psB,
        ):
            # persistent activations (features on partitions, 2 chunks of 128).
            # The key/value sequence is processed in per-core order
            # [query-half, other-half] — attention's j index is a pure
            # reduction index (sigmoid, not softmax), so any consistent
            # permutation of key rows gives the same output.
            # xq on sync (QT needs it immediately); xo on gpsimd so it
            # loads in parallel with the weights.
            xq = []
            for kk in range(2):
                t = const.tile([128, SQ], bdt, name=f"xq{kk}", tag=f"xq{kk}")
                nc.sync.dma_start(out=t[:], in_=xqT[kk * 128 : (kk + 1) * 128, :])
                xq.append(t)
            xo = []
            xo_dmas = []
            for kk in range(2):
                t = const.tile([128, SQ], bdt, name=f"xo{kk}", tag=f"xo{kk}")
                d = nc.gpsimd.dma_start(out=t[:], in_=xoT[kk * 128 : (kk + 1) * 128, :])
                xo.append(t)
                xo_dmas.append(d)
            xhalves = [xq, xo]

            # PE clock warmup: junk matmuls while the first DMAs land, so
            # the Tensor engine is at full pstate when real work arrives.
            # memset on gpsimd: its preamble clears ~1.5us before vector's.
            junk = const.tile([128, SQ], bdt, name="junk", tag="junk")
            nc.gpsimd.memset(junk[:], 0.0)
            wps = psA.tile([128, SQ], f32, name="warm", tag="psA")
            for _ in range(N_WARM):
                nc.tensor.matmul(
                    wps[:], junk[:, :128], junk[:], start=True, stop=True
                )

            # output accumulated directly in PSUM across all 8 heads: the
            # per-head O-projection matmuls form one start/stop group per
            # out tile (2 banks hold the 4 [128,256] query-row tiles).
            pso_out = [
                psB.tile([128, 512], f32, name=f"psout{i}", tag=f"psO{i}", bufs=1)
                for i in range(2)
            ]
            out_acc = [pso_out[m // 2][:, (m % 2) * F : (m % 2 + 1) * F] for m in range(NM)]

            prev = None
            for h in range(H):
                # head weights: one packed DMA per queue per head. Queue
                # FIFO order matches consumption order: sync carries wq+wk
                # (QT -> KT phases); gpsimd carries xo (KT needs it at
                # h==0) then wv+wo.
                t_qk = sb.tile([128, 4 * F], bdt, name="wqk", tag="wqk", bufs=4)
                d_qk = nc.sync.dma_start(out=t_qk[:], in_=Wqk[h])
                if h == 0:
                    # startup HBM burst: give the first matmul's inputs
                    # (xq + Wqk on sync) the full bandwidth; xo (and the
                    # gpsimd queue behind it) waits until Wqk has landed
                    add_dep_helper(xo_dmas[0].ins, d_qk.ins, reason="hbm priority")
                t_vo = sb.tile([128, 4 * F], bdt, name="wvo", tag="wvo", bufs=4)
                nc.gpsimd.dma_start(out=t_vo[:], in_=Wvo[h])
                wq = [t_qk[:, 0:F], t_qk[:, F : 2 * F]]
                wk = [t_qk[:, 2 * F : 3 * F], t_qk[:, 3 * F : 4 * F]]
                wv = [t_vo[:, 0:F], t_vo[:, F : 2 * F]]
                wo = [t_vo[:, 2 * F : 3 * F], t_vo[:, 3 * F : 4 * F]]

                # QT_h [256 fh, 512 q] as 2 tiles [128, 512]
                qT = []
                for m in range(2):
                    ps = psA.tile([128, SQ], f32, name="psq", tag="psA")
                    for kk in range(2):
                        nc.tensor.matmul(
                            ps[:],
                            wq[kk][:, m * 128 : (m + 1) * 128],
                            xq[kk][:],
                            start=(kk == 0),
                            stop=(kk == 1),
                        )
                    t = sb.tile([128, SQ], bdt, name=f"qT{m}", tag=f"qT{m}")
                    nc.vector.tensor_copy(t[:], ps[:])
                    qT.append(t)

                # deferred output projection of the PREVIOUS head (gives the
                # DVE time to finish the ot casts without stalling the PE),
                # accumulated directly into the persistent PSUM out banks.
                # start=True only on the first matmul touching each bank
                # (the has_written reset is bank-granular).
                if prev is not None:
                    p_ot, p_wo, p_h = prev
                    for m in range(NM):
                        for kk in range(2):
                            nc.tensor.matmul(
                                out_acc[m],
                                p_ot[kk][:, m * 128 : (m + 1) * 128],
                                p_wo[kk],
                                start=(p_h == 0 and m % 2 == 0 and kk == 0),
                                stop=False,
                                skip_group_check=True,
                            )

                # KT_h [256 fh, 1024 j] as 2 tiles [128, 1024]
                # (j order = [query-half rows, other-half rows])
                kT = []
                for m in range(2):
                    t = sb.tile([128, S], bdt, name=f"kT{m}", tag=f"kT{m}")
                    for n in range(2):
                        ps = psA.tile([128, 512], f32, name="psk", tag="psA")
                        for kk in range(2):
                            nc.tensor.matmul(
                                ps[:],
                                wk[kk][:, m * 128 : (m + 1) * 128],
                                xhalves[n][kk][:],
                                start=(kk == 0),
                                stop=(kk == 1),
                            )
                        if n == 0:
                            nc.vector.tensor_copy(
                                t[:, n * 512 : (n + 1) * 512], ps[:]
                            )
                        else:
                            nc.scalar.copy(
                                t[:, n * 512 : (n + 1) * 512], ps[:]
                            )
                    kT.append(t)

                # V_h [1024 j, 256 fh] as 4 PAIR tiles [128, 2, 256] fp8
                # (j=2p and j=2p+1 share a tile for DoubleRow contraction)
                v = []
                for p in range(NP):
                    t = sb.tile([128, 2, F], f8, name=f"v{p}", tag=f"v{p}")
                    v.append(t)
                for j in range(NJ):
                    ps = psA.tile([128, F], f32, name="psv", tag="psA")
                    xh = xhalves[j // 4]
                    jj = j % 4
                    for kk in range(2):
                        nc.tensor.matmul(
                            ps[:],
                            xh[kk][:, jj * 128 : (jj + 1) * 128],
                            wv[kk],
                            start=(kk == 0),
                            stop=(kk == 1),
                        )
                    nc.vector.tensor_copy(v[j // 2][:, j % 2], ps[:])

                # ST_j = [128 j, 512 q] -> sigmoid(16*ST) -> fp8 AT pair
                # tiles [128, 2, 512]; the OT accumulation
                # (OT_h = sum_p Vpair_p^T-block @ ATpair_p, DoubleRow)
                # is software-pipelined behind the sigmoid ACTs.
                at = [
                    sb.tile([128, 2, SQ], f8, name=f"at{p}", tag=f"at{p}")
                    for p in range(NP)
                ]
                pso = [
                    psB.tile([128, SQ], f32, name=f"pso{m}", tag=f"psB{m}", bufs=1)
                    for m in range(2)
                ]

                def o_contrib(p):
                    for m in range(2):
                        nc.tensor.matmul(
                            pso[m][:],
                            v[p][:, :, m * 128 : (m + 1) * 128],
                            at[p][:],
                            start=(p == 0),
                            stop=(p == NP - 1),
                            perf_mode=DoubleRow,
                        )

                for j in range(NJ):
                    ps = psA.tile([128, SQ], f32, name="pss", tag="psA")
                    for kk in range(2):
                        nc.tensor.matmul(
                            ps[:],
                            kT[kk][:, j * 128 : (j + 1) * 128],
                            qT[kk][:],
                            start=(kk == 0),
                            stop=(kk == 1),
                        )
                    nc.scalar.activation(at[j // 2][:, j % 2], ps[:], Sigmoid, scale=SCALE)
                    # lag the OT accumulation two pairs behind the scores so
                    # the last pair's sigmoid hides under the p2+p3 matmuls
                    if j >= 5 and j % 2 == 1:
                        o_contrib((j - 5) // 2)
                o_contrib(NP - 2)
                o_contrib(NP - 1)

                ot = []
                for m in range(2):
                    t = sb.tile([128, SQ], bdt, name=f"ot{m}", tag=f"ot{m}")
                    # final head: split the two casts across DVE and ACT so
                    # the closing O-projection isn't gated on serial DVE
                    eng = nc.scalar if (h == H - 1 and m == 1) else nc.vector
                    if eng is nc.scalar:
                        eng.copy(t[:], pso[m][:])
                    else:
                        eng.tensor_copy(t[:], pso[m][:])
                    ot.append(t)

                prev = (ot, wo, h)

            # final head's output projection closes the PSUM accumulation;
            # relu (reading PSUM directly) + store per tile as soon as that
            # tile's group closes
            p_ot, p_wo, p_h = prev
            for m in range(NM):
                for kk in range(2):
                    nc.tensor.matmul(
                        out_acc[m],
                        p_ot[kk][:, m * 128 : (m + 1) * 128],
                        p_wo[kk],
                        start=False,
                        stop=(kk == 1),
                        skip_group_check=True,
                    )
                t = osb.tile([128, F], f32, name=f"outsb{m}", tag=f"outsb{m}")
                # relu split across ACT and DVE so the 4 tiles don't
                # serialize on one engine in the tail
                if m % 2 == 0:
                    nc.scalar.activation(t[:], out_acc[m], Relu)
                else:
                    nc.vector.tensor_scalar_max(t[:], out_acc[m], 0.0)
                eng = nc.sync if m % 2 == 0 else nc.gpsimd
                eng.dma_start(out=out_d[m * 128 : (m + 1) * 128, :], in_=t[:])

    nc.finalize()
    return nc


def _get_nc():
    if "nc" not in _CACHE:
        _CACHE["nc"] = _build_nc()
    return _CACHE["nc"]


def _pack_kk(w):
    # [H, F, F] -> [H, 128, 2F]: the two 128-row input-feature chunks side
    # by side in the free dim (one DMA per head covers both matmul chunks)
    return w.reshape(H, 2, 128, F).transpose(0, 2, 1, 3).reshape(H, 128, 2 * F)


def _prep_weights(Wq, Wk, Wv, Wo):
    # [F, F*H] with column f*H+h  ->  [H, F, F] head-contiguous, bf16
    wq = Wq.reshape(F, F, H).transpose(2, 0, 1).astype(BF16)
    wk = Wk.reshape(F, F, H).transpose(2, 0, 1).astype(BF16)
    wv = Wv.reshape(F, F, H).transpose(2, 0, 1).astype(BF16)
    # [F*H, F] with row f*H+h  ->  [H, F, F]
    wo = Wo.reshape(F, H, F).transpose(1, 0, 2).astype(BF16)
    wqk = np.ascontiguousarray(np.concatenate([_pack_kk(wq), _pack_kk(wk)], axis=2))
    wvo = np.ascontiguousarray(np.concatenate([_pack_kk(wv), _pack_kk(wo)], axis=2))
    return wqk, wvo


def kernel(q_input, Wq, Wk, Wv, Wo, _trace=False):
    from concourse.bass_utils import run_bass_kernel_spmd

    nc = _get_nc()
    wqk, wvo = _prep_weights(
        np.asarray(Wq, np.float32),
        np.asarray(Wk, np.float32),
        np.asarray(Wv, np.float32),
        np.asarray(Wo, np.float32),
    )
    q_input = np.asarray(q_input, np.float32)

    in_maps = []
    for c in range(N_CORES):
        b, half = c // 2, c % 2
        xT = q_input[b].T
        xqT = np.ascontiguousarray(xT[:, half * SQ : (half + 1) * SQ]).astype(BF16)
        xoT = np.ascontiguousarray(xT[:, (1 - half) * SQ : (2 - half) * SQ]).astype(BF16)
        in_maps.append({"xqT": xqT, "xoT": xoT, "Wqk": wqk, "Wvo": wvo})

    res = run_bass_kernel_spmd(nc, in_maps, list(range(N_CORES)), trace=_trace)

    out = np.empty((B, S, F), np.float32)
    for c in range(N_CORES):
        b, half = c // 2, c % 2
        out[b, half * SQ : (half + 1) * SQ, :] = res.results[c]["out"]
    if _trace:
        return out, res
    return out


# revision 51
# speedup vs baseline: 1.0430x; 1.0430x over previous
"""Self-attention (sigmoid attention) Bass kernel for Trainium2, SPMD on 8 cores.

Problem: B=4, S=1024, F=256, H=8
  q = (X @ Wq).reshape(b,s,f,h); k,v likewise (self-attention)
  attn = sigmoid(sqrt(F) * q.kT) per (b,h);  wv = attn @ v
  out = relu(wv_flat @ Wo)

Sharding: data-parallel over (batch, seq-half): core c handles batch c//2,
query rows [half*512, half*512+512). K/V are computed per-core for the full
batch sequence (duplicated across the 2 cores sharing a batch) — no
collectives needed.

Precision plan (validated vs fp32 reference, rel-max tolerance 2e-2):
  - all activations/weights bf16 (rel err 6e-3); same PE rate as fp32r but
    half the DMA/DVE/SBUF traffic and Fast Weight Load on LDWEIGHTS.
  - the attn@V reduction runs in fp8-e4m3 with MatmulPerfMode.DoubleRow
    (two j-tiles packed per matmul -> 2x ALU throughput); adds ~1e-2 rel
    err, total ~1.6e-2. PSUM accumulation is fp32 throughout.

Per-core layout (head-contiguous permuted weights, prepared on host):
  xqT  [256, 512]  X[b]^T query-half columns (features on partitions)
  xoT  [256, 512]  other-half columns
  Wq/Wk/Wv [8,256,256] (h, f_in, f_head):  W[:, f*H+h] -> [h, :, f]
  Wo   [8,256,256] (h, f_head, n):         Wo[f*H+h, :] -> [h, f, :]
Pipeline per head h: QT=[wq^T x_q], KT, V via projection matmuls;
ST_j = KT_j^T-block @ QT (psum) -> sigmoid*16 -> fp8 AT pair tiles;
OT = sum_jp Vpair_jp^T @ ATpair_jp (DoubleRow); out += OT^T @ Wo_h
accumulated in persistent PSUM banks across heads; relu at the end.
"""

import numpy as np
import ml_dtypes

B, S, F, H = 4, 1024, 256, 8
N_CORES = 8
SCALE = 16.0  # sqrt(F)
SQ = S // 2  # query rows per core

BF16 = ml_dtypes.bfloat16

N_WARM = 10  # PE clock-warmup matmuls issued during the initial DMA wait

_CACHE = {}


def _build_nc():
    import concourse.mybir as mybir
    import concourse.tile as tile
    from concourse import bacc

    f32 = mybir.dt.float32
    bdt = mybir.dt.bfloat16
    f8 = mybir.dt.float8e4
    DoubleRow = mybir.MatmulPerfMode.DoubleRow

    Sigmoid = mybir.ActivationFunctionType.Sigmoid
    Relu = mybir.ActivationFunctionType.Relu

    nc = bacc.Bacc()
    xqT = nc.declare_dram_parameter("xqT", [F, SQ], bdt, isOutput=False)
    xoT = nc.declare_dram_parameter("xoT", [F, SQ], bdt, isOutput=False)
    # weights packed per head as [128, 4*F] = [w?0 | w?1 | w?0 | w?1]
    # (kk chunks side by side) so each head needs one DMA per queue
    Wqk = nc.declare_dram_parameter("Wqk", [H, 128, 4 * F], bdt, isOutput=False)
    Wvo = nc.declare_dram_parameter("Wvo", [H, 128, 4 * F], bdt, isOutput=False)
    out_d = nc.declare_dram_parameter("out", [SQ, F], f32, isOutput=True)

    NJ = S // 128  # 8 key-row tiles per head
    NP = NJ // 2  # 4 key-row tile PAIRS (fp8 DoubleRow granularity)
    NM = SQ // 128  # 4 query-row tiles

    with tile.TileContext(nc) as tc:
        with (
            tc.tile_pool(name="const", bufs=1) as const,
            tc.tile_pool(name="sb", bufs=2) as sb,
            tc.tile_pool(name="osb", bufs=1) as osb,
            tc.tile_pool(name="psA", bufs=4, space="PSUM") as psA,
            tc.tile_pool(name="psB", bufs=2, space="PSUM") as psB,
        ):
            # persistent activations (features on partitions, 2 chunks of 128).
            # The key/value sequence is processed in per-core order
            # [query-half, other-half] — attention's j index is a pure
            # reduction index (sigmoid, not softmax), so any consistent
            # permutation of key rows gives the same output.
            # xq on sync (QT needs it immediately); xo on gpsimd so it
            # loads in parallel with the weights.
            xq = []
            for kk in range(2):
                t = const.tile([128, SQ], bdt, name=f"xq{kk}", tag=f"xq{kk}")
                nc.sync.dma_start(out=t[:], in_=xqT[kk * 128 : (kk + 1) * 128, :])
                xq.append(t)
            xo = []
            for kk in range(2):
                t = const.tile([128, SQ], bdt, name=f"xo{kk}", tag=f"xo{kk}")
                nc.gpsimd.dma_start(out=t[:], in_=xoT[kk * 128 : (kk + 1) * 128, :])
                xo.append(t)
            xhalves = [xq, xo]

            # PE clock warmup: junk matmuls while the first DMAs land, so
            # the Tensor engine is at full pstate when real work arrives.
            # memset on gpsimd: its preamble clears ~1.5us before vector's.
            junk = const.tile([128, SQ], bdt, name="junk", tag="junk")
            nc.gpsimd.memset(junk[:], 0.0)
            wps = psA.tile([128, SQ], f32, name="warm", tag="psA")
            for _ in range(N_WARM):
                nc.tensor.matmul(
                    wps[:], junk[:, :128], junk[:], start=True, stop=True
                )

            # output accumulated directly in PSUM across all 8 heads: the
            # per-head O-projection matmuls form one start/stop group per
            # out tile (2 banks hold the 4 [128,256] query-row tiles).
            pso_out = [
                psB.tile([128, 512], f32, name=f"psout{i}", tag=f"psO{i}", bufs=1)
                for i in range(2)
            ]
            out_acc = [pso_out[m // 2][:, (m % 2) * F : (m % 2 + 1) * F] for m in range(NM)]

            prev = None
            for h in range(H):
                # head weights: one packed DMA per queue per head. Queue
                # FIFO order matches consumption order: sync carries wq+wk
                # (QT -> KT phases); gpsimd carries xo (KT needs it at
                # h==0) then wv+wo.
                t_qk = sb.tile([128, 4 * F], bdt, name="wqk", tag="wqk", bufs=4)
                nc.sync.dma_start(out=t_qk[:], in_=Wqk[h])
                t_vo = sb.tile([128, 4 * F], bdt, name="wvo", tag="wvo", bufs=4)
                nc.gpsimd.dma_start(out=t_vo[:], in_=Wvo[h])
                wq = [t_qk[:, 0:F], t_qk[:, F : 2 * F]]
                wk = [t_qk[:, 2 * F : 3 * F], t_qk[:, 3 * F : 4 * F]]
                wv = [t_vo[:, 0:F], t_vo[:, F : 2 * F]]
                wo = [t_vo[:, 2 * F : 3 * F], t_vo[:, 3 * F : 4 * F]]

                # QT_h [256 fh, 512 q] as 2 tiles [128, 512]
                qT = []
                for m in range(2):
                    ps = psA.tile([128, SQ], f32, name="psq", tag="psA")
                    for kk in range(2):
                        nc.tensor.matmul(
                            ps[:],
                            wq[kk][:, m * 128 : (m + 1) * 128],
                            xq[kk][:],
                            start=(kk == 0),
                            stop=(kk == 1),
                        )
                    t = sb.tile([128, SQ], bdt, name=f"qT{m}", tag=f"qT{m}")
                    nc.vector.tensor_copy(t[:], ps[:])
                    qT.append(t)

                # deferred output projection of the PREVIOUS head (gives the
                # DVE time to finish the ot casts without stalling the PE),
                # accumulated directly into the persistent PSUM out banks.
                # start=True only on the first matmul touching each bank
                # (the has_written reset is bank-granular).
                if prev is not None:
                    p_ot, p_wo, p_h = prev
                    for m in range(NM):
                        for kk in range(2):
                            nc.tensor.matmul(
                                out_acc[m],
                                p_ot[kk][:, m * 128 : (m + 1) * 128],
                                p_wo[kk],
                                start=(p_h == 0 and m % 2 == 0 and kk == 0),
                                stop=False,
                                skip_group_check=True,
                            )

                # KT_h [256 fh, 1024 j] as 2 tiles [128, 1024]
                # (j order = [query-half rows, other-half rows])
                kT = []
                for m in range(2):
                    t = sb.tile([128, S], bdt, name=f"kT{m}", tag=f"kT{m}")
                    for n in range(2):
                        ps = psA.tile([128, 512], f32, name="psk", tag="psA")
                        for kk in range(2):
                            nc.tensor.matmul(
                                ps[:],
                                wk[kk][:, m * 128 : (m + 1) * 128],
                                xhalves[n][kk][:],
                                start=(kk == 0),
                                stop=(kk == 1),
                            )
                        if n == 0:
                            nc.vector.tensor_copy(
                                t[:, n * 512 : (n + 1) * 512], ps[:]
                            )
                        else:
                            nc.scalar.copy(
                                t[:, n * 512 : (n + 1) * 512], ps[:]
                            )
                    kT.append(t)

                # V_h [1024 j, 256 fh] as 4 PAIR tiles [128, 2, 256] fp8
                # (j=2p and j=2p+1 share a tile for DoubleRow contraction)
                v = []
                for p in range(NP):
                    t = sb.tile([128, 2, F], f8, name=f"v{p}", tag=f"v{p}")
                    v.append(t)
                for j in range(NJ):
                    ps = psA.tile([128, F], f32, name="psv", tag="psA")
                    xh = xhalves[j // 4]
                    jj = j % 4
                    for kk in range(2):
                        nc.tensor.matmul(
                            ps[:],
                            xh[kk][:, jj * 128 : (jj + 1) * 128],
                            wv[kk],
                            start=(kk == 0),
                            stop=(kk == 1),
                        )
                    nc.vector.tensor_copy(v[j // 2][:, j % 2], ps[:])

                # ST_j = [128 j, 512 q] -> sigmoid(16*ST) -> fp8 AT pair
                # tiles [128, 2, 512]; the OT accumulation
                # (OT_h = sum_p Vpair_p^T-block @ ATpair_p, DoubleRow)
                # is software-pipelined behind the sigmoid ACTs.
                at = [
                    sb.tile([128, 2, SQ], f8, name=f"at{p}", tag=f"at{p}")
                    for p in range(NP)
                ]
                pso = [
                    psB.tile([128, SQ], f32, name=f"pso{m}", tag=f"psB{m}", bufs=1)
                    for m in range(2)
                ]

                def o_contrib(p):
                    for m in range(2):
                        nc.tensor.matmul(
                            pso[m][:],
                            v[p][:, :, m * 128 : (m + 1) * 128],
                            at[p][:],
                            start=(p == 0),
                            stop=(p == NP - 1),
                            perf_mode=DoubleRow,
                        )

                for j in range(NJ):
                    ps = psA.tile([128, SQ], f32, name="pss", tag="psA")
                    for kk in range(2):
                        nc.tensor.matmul(
                            ps[:],
                            kT[kk][:, j * 128 : (j + 1) * 128],
                            qT[kk][:],
                            start=(kk == 0),
                            stop=(kk == 1),
                        )
                    nc.scalar.activation(at[j // 2][:, j % 2], ps[:], Sigmoid, scale=SCALE)
                    # lag the OT accumulation two pairs behind the scores so
                    # the last pair's sigmoid hides under the p2+p3 matmuls
                    if j >= 5 and j % 2 == 1:
                        o_contrib((j - 5) // 2)
                o_contrib(NP - 2)
                o_contrib(NP - 1)

                ot = []
                for m in range(2):
                    t = sb.tile([128, SQ], bdt, name=f"ot{m}", tag=f"ot{m}")
                    # final head: split the two casts across DVE and ACT so
                    # the closing O-projection isn't gated on serial DVE
                    eng = nc.scalar if (h == H - 1 and m == 1) else nc.vector
                    if eng is nc.scalar:
                        eng.copy(t[:], pso[m][:])
                    else:
                        eng.tensor_copy(t[:], pso[m][:])
                    ot.append(t)

                prev = (ot, wo, h)

            # final head's output projection closes the PSUM accumulation;
            # relu (reading PSUM directly) + store per tile as soon as that
            # tile's group closes
            p_ot, p_wo, p_h = prev
            for m in range(NM):
                for kk in range(2):
                    nc.tensor.matmul(
                        out_acc[m],
                        p_ot[kk][:, m * 128 : (m + 1) * 128],
                        p_wo[kk],
                        start=False,
                        stop=(kk == 1),
                        skip_group_check=True,
                    )
                t = osb.tile([128, F], f32, name=f"outsb{m}", tag=f"outsb{m}")
                # relu split across ACT and DVE so the 4 tiles don't
                # serialize on one engine in the tail
                if m % 2 == 0:
                    nc.scalar.activation(t[:], out_acc[m], Relu)
                else:
                    nc.vector.tensor_scalar_max(t[:], out_acc[m], 0.0)
                eng = nc.sync if m % 2 == 0 else nc.gpsimd
                eng.dma_start(out=out_d[m * 128 : (m + 1) * 128, :], in_=t[:])

    nc.finalize()
    return nc


def _get_nc():
    if "nc" not in _CACHE:
        _CACHE["nc"] = _build_nc()
    return _CACHE["nc"]


def _pack_kk(w):
    # [H, F, F] -> [H, 128, 2F]: the two 128-row input-feature chunks side
    # by side in the free dim (one DMA per head covers both matmul chunks)
    return w.reshape(H, 2, 128, F).transpose(0, 2, 1, 3).reshape(H, 128, 2 * F)


def _prep_weights(Wq, Wk, Wv, Wo):
    # [F, F*H] with column f*H+h  ->  [H, F, F] head-contiguous, bf16
    wq = Wq.reshape(F, F, H).transpose(2, 0, 1).astype(BF16)
    wk = Wk.reshape(F, F, H).transpose(2, 0, 1).astype(BF16)
    wv = Wv.reshape(F, F, H).transpose(2, 0, 1).astype(BF16)
    # [F*H, F] with row f*H+h  ->  [H, F, F]
    wo = Wo.reshape(F, H, F).transpose(1, 0, 2).astype(BF16)
    wqk = np.ascontiguousarray(np.concatenate([_pack_kk(wq), _pack_kk(wk)], axis=2))
    wvo = np.ascontiguousarray(np.concatenate([_pack_kk(wv), _pack_kk(wo)], axis=2))
    return wqk, wvo


def kernel(q_input, Wq, Wk, Wv, Wo, _trace=False):
    from concourse.bass_utils import run_bass_kernel_spmd

    nc = _get_nc()
    wqk, wvo = _prep_weights(
        np.asarray(Wq, np.float32),
        np.asarray(Wk, np.float32),
        np.asarray(Wv, np.float32),
        np.asarray(Wo, np.float32),
    )
    q_input = np.asarray(q_input, np.float32)

    in_maps = []
    for c in range(N_CORES):
        b, half = c // 2, c % 2
        xT = q_input[b].T
        xqT = np.ascontiguousarray(xT[:, half * SQ : (half + 1) * SQ]).astype(BF16)
        xoT = np.ascontiguousarray(xT[:, (1 - half) * SQ : (2 - half) * SQ]).astype(BF16)
        in_maps.append({"xqT": xqT, "xoT": xoT, "Wqk": wqk, "Wvo": wvo})

    res = run_bass_kernel_spmd(nc, in_maps, list(range(N_CORES)), trace=_trace)

    out = np.empty((B, S, F), np.float32)
    for c in range(N_CORES):
        b, half = c // 2, c % 2
        out[b, half * SQ : (half + 1) * SQ, :] = res.results[c]["out"]
    if _trace:
        return out, res
    return out
